# revision 2
# baseline (speedup 1.0000x reference)
"""MiniBindingAttention Trainium2 kernel (v2).

Reference computation (per batch b, head h, T=2048, HD=64):
    Q = x_h * sign(bv_q); K = x_h * sign(bv_k); V = x_h * sign(bv_v)
    scores = Q @ K.T / sqrt(HD)
    attn   = causal ? sigmoid(4 * scores) : 0
    out    = attn @ V

Key algebra / layout:
  - sigmoid(4*scale*QK) = sigmoid((x_q . x_k) * 0.5 * sq*sk); the per-channel
    factor 0.5*sign(bv_q)*sign(bv_k) is folded into one scaled copy of x^T
    (wxT) on the host; sign(bv_v) is folded into the host-prepared xN, so the
    second matmul needs no sign fixup at all.
  - scores are computed TRANSPOSED ([k, q]); x^T and wxT are stored duplicated
    along partitions (0:64 / 64:128) so two k-tiles' score matmuls co-execute
    in disjoint PE row groups (one 512-cycle stream covers 2 k-tiles).
  - one interleaved dram tensor xw holds per-512-col chunks [xT_c | wxT_c] so
    each chunk is a single dma_start (sync-queue DIRECT2D issue is ~650ns
    each and paced the old kernel's startup).
  - sigmoid runs on the scalar engine in large per-group instructions
    (PSUM groups of 4 k-tiles x 512 q / 2 k-tiles x 512 q); two groups per
    pair are instead approximated on the otherwise-idle vector engine with a
    2-clamp piecewise-linear sigmoid (max err 0.017, end-to-end rel err
    ~2.7e-3), splitting the elementwise work across two engines.
  - causal masking: gpsimd affine_select zeroes the lower staircase of the
    diagonal 128x128 blocks in one strided instruction per group.
  - mm2 accumulates all k-tiles of a q-block into a single [64,512] PSUM
    region; a DVE copy moves it to an SBUF staging tile, one output DMA per
    pair.

Sharding: B*H = 32 (batch, head) pairs, 4 per core across 8 cores.
"""

import numpy as np
import ml_dtypes

import concourse.tile as tile
from concourse import bacc, mybir
from concourse.bass_utils import run_bass_kernel_spmd

N_CORES = 8
B, T, D, H, HD = 2, 2048, 1024, 16, 64
PAIRS = (B * H) // N_CORES  # 4 (b,h) pairs per core
KT = T // 128               # 16 k-tiles of 128 rows
QB = T // 512               # 4 q-blocks of 512 cols
F32 = mybir.dt.float32
F32R = mybir.dt.float32r
BF16 = mybir.dt.bfloat16
SIG = mybir.ActivationFunctionType.Sigmoid
MULT = mybir.AluOpType.mult
ADD = mybir.AluOpType.add
MIN = mybir.AluOpType.min
MAX = mybir.AluOpType.max
GE = mybir.AluOpType.is_ge

# groups per q-block j: (buf, [ktiles]); buf A = [128,2048] psum, B = [128,1024]
GROUPS = {
    0: [("A", [0, 1, 2, 3])],
    1: [("B", [0, 1]), ("A", [2, 3, 4, 5]), ("B", [6, 7])],
    2: [("A", [0, 1, 2, 3]), ("B", [4, 5]), ("A", [6, 7, 8, 9]),
        ("B", [10, 11])],
    3: [("A", [0, 1, 2, 3]), ("B", [4, 5]), ("A", [6, 7, 8, 9]),
        ("B", [10, 11]), ("A", [12, 13, 14, 15])],
}
# groups whose sigmoid runs as a PWL approximation on the vector engine
OFFLOAD = {(2, 0), (3, 0)}

# 2-clamp PWL sigmoid: sig(z) ~ 0.5 + clip(A1*z,+-B1) + clip(A2*z,+-B2),
# constrained B1+B2=0.5 so the tails saturate exactly (no systematic bias).
PA1, PB1, PA2, PB2 = 0.060167, 0.242785, 0.155614, 0.257215


def _round_fp32r(a: np.ndarray) -> np.ndarray:
    """Round fp32 to the hardware fp32r format (11-bit mantissa, RNE)."""
    v = a.astype(np.float32).view(np.uint32).astype(np.uint64)
    r = (v + 0x7FF + ((v >> 12) & 1)) & 0xFFFFF000
    return r.astype(np.uint32).view(np.float32)


def _wxcol(i: int) -> int:
    """Column of wxT k-tile i inside the interleaved xw layout."""
    return 1024 * (i // 4) + 512 + 128 * (i % 4)


def build():
    nc = bacc.Bacc("TRN2", target_bir_lowering=False)
    # xw: per 512-col chunk c: [xT cols 512c:512c+512 | wxT same cols], both
    # duplicated along partitions (0:64 == 64:128)
    xw_d = nc.dram_tensor("xw", [PAIRS, 128, 2 * T], F32R, kind="ExternalInput")
    # xN pre-swizzled + sign(bv_v)-folded: xn[pp, 64k+d] = x[128k+pp, d]*sv[d]
    xn_d = nc.dram_tensor("xn", [PAIRS, 128, KT * HD], BF16, kind="ExternalInput")
    out_d = nc.dram_tensor("outT", [PAIRS, HD, T], F32, kind="ExternalOutput")

    with tile.TileContext(nc) as tc:
        with (
            tc.tile_pool(name="xwp", bufs=2) as xwp,
            tc.tile_pool(name="xnp", bufs=2) as xnp,
            tc.tile_pool(name="attap", bufs=3) as attap,
            tc.tile_pool(name="attbp", bufs=2) as attbp,
            tc.tile_pool(name="pwlp", bufs=2) as pwlp,
            tc.tile_pool(name="outp", bufs=2) as outp,
            tc.tile_pool(name="psa", bufs=1, space="PSUM") as psa,
            tc.tile_pool(name="psb", bufs=1, space="PSUM") as psb,
            tc.tile_pool(name="pso", bufs=2, space="PSUM") as pso,
        ):
            zero_reg = nc.gpsimd.to_reg(0.0)
            state = {}

            def load_pair(p):
                xw = xwp.tile([128, 2 * T], F32R, name="xw", tag="xw")
                xn = xnp.tile([128, KT * HD], BF16, name="xn", tag="xn")
                nc.sync.dma_start(out=xw[:, 0:1024], in_=xw_d[p, :, 0:1024])
                nc.sync.dma_start(out=xn, in_=xn_d[p])
                for c in range(1, 4):
                    cs = slice(1024 * c, 1024 * c + 1024)
                    nc.sync.dma_start(out=xw[:, cs], in_=xw_d[p, :, cs])
                state[p] = (xw, xn)

            def emit_mm2(g):
                """Accumulate one group's k-tiles into the j-block oacc."""
                p, j, att, kts, oacc, mmpos = g
                _, xn = state[p]
                n_total = 4 * j + 4
                for slot, i in enumerate(kts):
                    r = i - 4 * j
                    o0 = 128 * r if r >= 1 else 0
                    nc.tensor.matmul(
                        out=oacc[:, o0:512],
                        lhsT=xn[:, HD * i : HD * i + HD],
                        rhs=att[:, 512 * slot + o0 : 512 * slot + 512],
                        start=(mmpos[0] == 0),
                        stop=(mmpos[0] == n_total - 1),
                    )
                    mmpos[0] += 1

            # flat schedule
            sched = [
                (p, j, gi, buf, kts)
                for p in range(PAIRS)
                for j in range(QB)
                for gi, (buf, kts) in enumerate(GROUPS[j])
            ]

            load_pair(0)
            pend = []      # same-j groups awaiting mm2 (pipeline depth 1)
            flush = None   # deferred end-of-j work: (pend+defr list, oacc,
                           # out_sb, j, p, is_pair_end)
            defr = []      # PWL groups' mm2, deferred to end of j
            out_sb = None
            oacc = None
            mmpos = None

            for (p, j, gi, buf, kts) in sched:
                if gi == 0:
                    if p + 1 < PAIRS and j == 2 and p + 1 not in state:
                        load_pair(p + 1)
                    if j == 0:
                        out_sb_next = outp.tile([HD, T], F32, name="osb", tag="osb")
                    else:
                        out_sb_next = out_sb
                    oacc_next = pso.tile([HD, 512], F32, name="oacc", tag="oacc")
                    mmpos_next = [0]

                xw, xn = state[p]
                # --- score matmuls for this group
                if gi == 0:
                    S_pool, att_pool, NC_ = (psa, attap, 2048) if buf == "A" else (psb, attbp, 1024)
                else:
                    S_pool, att_pool, NC_ = (psa, attap, 2048) if buf == "A" else (psb, attbp, 1024)
                ncols = 512 * len(kts)
                S = S_pool.tile([128, 2048 if buf == "A" else 1024], F32,
                                name="S" + buf, tag="S" + buf)
                att = att_pool.tile([128, 2048 if buf == "A" else 1024], BF16,
                                    name="att" + buf, tag="att" + buf)
                act_ranges = []
                for pi in range(len(kts) // 2):
                    s0, s1 = 2 * pi, 2 * pi + 1
                    ia, ib = kts[s0], kts[s1]
                    r0 = ia - 4 * j
                    tr = 256 if r0 == 2 else 0
                    for sl, (i, s, t) in enumerate(((ia, s0, tr), (ib, s1, 0))):
                        bp = 64 * sl
                        nc.tensor.matmul(
                            out=S[:, 512 * s + t : 512 * s + 512],
                            lhsT=xw[bp : bp + 64, _wxcol(i) : _wxcol(i) + 128],
                            rhs=xw[bp : bp + 64, 1024 * j + t : 1024 * j + 512],
                            start=True,
                            stop=True,
                        )
                    a0 = 512 * s0 + tr
                    a1 = 512 * s1 + 512
                    if act_ranges and act_ranges[-1][1] == a0:
                        act_ranges[-1][1] = a1
                    else:
                        act_ranges.append([a0, a1])

                # --- end-of-previous-j flush sits here, AFTER this group's
                # score matmuls, so the PE has queued work while the old j
                # drains through act/mm2/copy.
                if flush is not None:
                    fl_groups, fl_oacc, fl_osb, fl_j, fl_p, pair_end = flush
                    for g in fl_groups:
                        emit_mm2(g)
                    nc.vector.tensor_scalar_mul(
                        fl_osb[:, 512 * fl_j : 512 * fl_j + 512], fl_oacc, 1.0
                    )
                    if pair_end:
                        nc.sync.dma_start(out=out_d[fl_p], in_=fl_osb)
                    flush = None
                if gi == 0:
                    out_sb = out_sb_next
                    oacc = oacc_next
                    mmpos = mmpos_next

                # --- sigmoid: scalar engine act(s) or DVE PWL chain
                offl = (j, gi) in OFFLOAD
                if offl:
                    zc = pwlp.tile([128, 2048], BF16, name="zc", tag="zc")
                    t1 = pwlp.tile([128, 2048], BF16, name="t1", tag="t1")
                    t2 = pwlp.tile([128, 2048], BF16, name="t2", tag="t2")
                    nc.vector.tensor_scalar(out=zc, in0=S[:, 0:ncols],
                                            scalar1=1.0, scalar2=None, op0=MULT)
                    nc.vector.tensor_scalar(out=t1, in0=zc, scalar1=PA1,
                                            scalar2=PB1, op0=MULT, op1=MIN)
                    nc.vector.tensor_scalar(out=t2, in0=zc, scalar1=PA2,
                                            scalar2=PB2, op0=MULT, op1=MIN)
                    nc.vector.tensor_scalar(out=t2, in0=t2, scalar1=-PB2,
                                            scalar2=0.5, op0=MAX, op1=ADD)
                    nc.vector.scalar_tensor_tensor(
                        out=att[:, 0:ncols], in0=t1, scalar=-PB1, in1=t2,
                        op0=MAX, op1=ADD)
                else:
                    for (a0, a1) in act_ranges:
                        nc.scalar.activation(out=att[:, a0:a1],
                                             in_=S[:, a0:a1], func=SIG)

                # --- causal staircase on diagonal blocks (gpsimd)
                diag = [(slot, i - 4 * j) for slot, i in enumerate(kts)
                        if i - 4 * j >= 0]
                if diag:
                    bi0 = 4 * diag[0][0] + diag[0][1]
                    n = len(diag)
                    v = att[:, 0:ncols].rearrange("p (n c) -> p n c", c=128)
                    sel = v[:, bi0 : bi0 + 5 * (n - 1) + 1 : 5, :]
                    nc.gpsimd.affine_select(
                        out=sel, in_=sel, pattern=[[0, n], [1, 128]],
                        compare_op=GE, fill=zero_reg, base=0,
                        channel_multiplier=-1)

                # --- pipeline: emit mm2 of the previous same-j group
                if pend:
                    emit_mm2(pend.pop(0))
                g = (p, j, att, kts, oacc, mmpos)
                if offl:
                    defr.append(g)
                else:
                    pend.append(g)

                # --- at end of j: defer the remaining mm2 + copy + (dma)
                last_of_j = gi == len(GROUPS[j]) - 1
                if last_of_j:
                    flush = (pend + defr, oacc, out_sb, j, p, j == QB - 1)
                    pend = []
                    defr = []

            # final flush
            fl_groups, fl_oacc, fl_osb, fl_j, fl_p, pair_end = flush
            for g in fl_groups:
                emit_mm2(g)
            nc.vector.tensor_scalar_mul(
                fl_osb[:, 512 * fl_j : 512 * fl_j + 512], fl_oacc, 1.0)
            nc.sync.dma_start(out=out_d[fl_p], in_=fl_osb)
    nc.compile()
    return nc


_CACHE: dict = {}


def _get_nc():
    if "nc" not in _CACHE:
        _CACHE["nc"] = build()
    return _CACHE["nc"]


def _make_in_maps(x, bv_q, bv_k, bv_v):
    x = np.asarray(x, dtype=np.float32)
    bv_q = np.asarray(bv_q, dtype=np.float32)
    bv_k = np.asarray(bv_k, dtype=np.float32)
    bv_v = np.asarray(bv_v, dtype=np.float32)
    w = 0.5 * np.sign(bv_q) * np.sign(bv_k)
    sv = np.sign(bv_v)

    in_maps = []
    for c in range(N_CORES):
        xw = np.empty((PAIRS, 128, 2 * T), np.float32)
        xn = np.empty((PAIRS, 128, KT * HD), ml_dtypes.bfloat16)
        for p in range(PAIRS):
            g = PAIRS * c + p
            b, h = divmod(g, H)
            xs = x[b, :, HD * h : HD * h + HD]  # [T, HD]
            xsT_r = _round_fp32r(xs.T)          # [HD, T]
            wxT = xsT_r * w[h][:, None]         # exact (+-0.5)
            for ch in range(4):
                cs = slice(512 * ch, 512 * ch + 512)
                xw[p, 0:HD, 1024 * ch : 1024 * ch + 512] = xsT_r[:, cs]
                xw[p, HD:128, 1024 * ch : 1024 * ch + 512] = xsT_r[:, cs]
                xw[p, 0:HD, 1024 * ch + 512 : 1024 * ch + 1024] = wxT[:, cs]
                xw[p, HD:128, 1024 * ch + 512 : 1024 * ch + 1024] = wxT[:, cs]
            xsv = xs * sv[h][None, :]
            xn[p] = (
                xsv.reshape(KT, 128, HD).transpose(1, 0, 2).reshape(128, KT * HD)
            )
        in_maps.append({"xw": xw, "xn": xn})
    return in_maps


def _assemble(results):
    out = np.empty((B, T, D), np.float32)
    for c in range(N_CORES):
        oT = results[c]["outT"]  # [PAIRS, HD, T]
        for p in range(PAIRS):
            g = PAIRS * c + p
            b, h = divmod(g, H)
            out[b, :, HD * h : HD * h + HD] = oT[p].T
    return out


def _run(x, bv_q, bv_k, bv_v, **spmd_kwargs):
    in_maps = _make_in_maps(x, bv_q, bv_k, bv_v)
    res = run_bass_kernel_spmd(
        _get_nc(), in_maps, core_ids=list(range(N_CORES)), **spmd_kwargs
    )
    return _assemble(res.results), res


def kernel(x, bv_q, bv_k, bv_v):
    out, _ = _run(x, bv_q, bv_k, bv_v)
    return out


# revision 4
# speedup vs baseline: 1.3651x; 1.3651x over previous
"""MiniBindingAttention Trainium2 kernel (v2.1).

Reference computation (per batch b, head h, T=2048, HD=64):
    Q = x_h * sign(bv_q); K = x_h * sign(bv_k); V = x_h * sign(bv_v)
    scores = Q @ K.T / sqrt(HD)
    attn   = causal ? sigmoid(4 * scores) : 0
    out    = attn @ V

Key algebra / layout:
  - sigmoid(4*scale*QK) = sigmoid((x_q . x_k) * 0.5 * sq*sk); the per-channel
    factor 0.5*sign(bv_q)*sign(bv_k) is folded into one scaled copy of x^T
    (wxT) on the host; sign(bv_v) is folded into the host-prepared xN, so the
    second matmul needs no sign fixup.
  - scores are computed TRANSPOSED ([k, q]); x^T and wxT are stored duplicated
    along partitions (0:64 / 64:128) so the two k-tiles of a wave co-execute
    in disjoint PE row groups (one 512-cycle stream covers both).
  - one interleaved dram tensor xw holds per-512-col chunks [xT_c | wxT_c] so
    each chunk is a single dma_start (sync-queue DIRECT2D issue is ~650ns
    each and paced the old kernel's startup).
  - per-wave units of [128,1024] PSUM with a 3-deep buffer pool keep the
    PE/ACT pipeline flowing; 4 waves per pair run their sigmoid as a 2-clamp
    piecewise-linear approximation on the vector engine instead (max err
    0.017, end-to-end rel err ~2.6e-3), splitting elementwise work across
    two engines.
  - causal masking: gpsimd affine_select zeroes the staircase of both
    diagonal 128x128 blocks of a wave in one strided instruction.
  - mm2 accumulates all k-tiles of a q-block into a single [64,512] PSUM
    region; a DVE copy moves it to an SBUF staging tile, one output DMA per
    pair.

Sharding: B*H = 32 (batch, head) pairs, 4 per core across 8 cores.
"""

import numpy as np
import ml_dtypes

import concourse.tile as tile
from concourse import bacc, mybir
from concourse.bass_utils import run_bass_kernel_spmd

N_CORES = 8
B, T, D, H, HD = 2, 2048, 1024, 16, 64
PAIRS = (B * H) // N_CORES  # 4 (b,h) pairs per core
KT = T // 128               # 16 k-tiles of 128 rows
QB = T // 512               # 4 q-blocks of 512 cols
F32 = mybir.dt.float32
F32R = mybir.dt.float32r
BF16 = mybir.dt.bfloat16
SIG = mybir.ActivationFunctionType.Sigmoid
MULT = mybir.AluOpType.mult
ADD = mybir.AluOpType.add
MIN = mybir.AluOpType.min
MAX = mybir.AluOpType.max
GE = mybir.AluOpType.is_ge

# waves whose sigmoid runs as a PWL approximation on the vector engine
OFFLOAD = {(2, 0), (2, 2), (3, 0), (3, 2)}

# 2-clamp PWL sigmoid: sig(z) ~ 0.5 + clip(A1*z,+-B1) + clip(A2*z,+-B2),
# constrained B1+B2=0.5 so the tails saturate exactly (no systematic bias).
PA1, PB1, PA2, PB2 = 0.060167, 0.242785, 0.155614, 0.257215


def _round_fp32r(a: np.ndarray) -> np.ndarray:
    """Round fp32 to the hardware fp32r format (11-bit mantissa, RNE)."""
    v = a.astype(np.float32).view(np.uint32).astype(np.uint64)
    r = (v + 0x7FF + ((v >> 12) & 1)) & 0xFFFFF000
    return r.astype(np.uint32).view(np.float32)


def _wxcol(i: int) -> int:
    """Column of wxT k-tile i inside the interleaved xw layout."""
    return 1024 * (i // 4) + 512 + 128 * (i % 4)


def build():
    nc = bacc.Bacc("TRN2", target_bir_lowering=False)
    # xw: per 512-col chunk c: [xT cols 512c:512c+512 | wxT same cols], both
    # duplicated along partitions (0:64 == 64:128)
    xw_d = nc.dram_tensor("xw", [PAIRS, 128, 2 * T], F32R, kind="ExternalInput")
    # xN pre-swizzled + sign(bv_v)-folded: xn[pp, 64k+d] = x[128k+pp, d]*sv[d]
    xn_d = nc.dram_tensor("xn", [PAIRS, 128, KT * HD], BF16, kind="ExternalInput")
    out_d = nc.dram_tensor("outT", [PAIRS, HD, T], F32, kind="ExternalOutput")

    with tile.TileContext(nc) as tc:
        with (
            tc.tile_pool(name="xwp", bufs=2) as xwp,
            tc.tile_pool(name="xnp", bufs=2) as xnp,
            tc.tile_pool(name="attnp", bufs=5) as attnp,
            tc.tile_pool(name="pwlp", bufs=2) as pwlp,
            tc.tile_pool(name="outp", bufs=2) as outp,
            tc.tile_pool(name="psum_s", bufs=3, space="PSUM") as psum_s,
            tc.tile_pool(name="psum_o", bufs=2, space="PSUM") as psum_o,
        ):
            zero_reg = nc.gpsimd.to_reg(0.0)
            state = {}

            def load_pair(p):
                xw = xwp.tile([128, 2 * T], F32R, name="xw", tag="xw")
                xn = xnp.tile([128, KT * HD], BF16, name="xn", tag="xn")
                nc.sync.dma_start(out=xw[:, 0:1024], in_=xw_d[p, :, 0:1024])
                nc.sync.dma_start(out=xn, in_=xn_d[p])
                for c in range(1, 4):
                    cs = slice(1024 * c, 1024 * c + 1024)
                    nc.sync.dma_start(out=xw[:, cs], in_=xw_d[p, :, cs])
                state[p] = (xw, xn)

            def emit_mm2(g):
                """Accumulate one wave's 2 k-tiles into the j-block oacc."""
                p, j, att, i0, oacc, mmpos = g
                _, xn = state[p]
                n_total = 4 * j + 4
                for s, i in enumerate((i0, i0 + 1)):
                    r = i - 4 * j
                    o0 = 128 * r if r >= 1 else 0
                    nc.tensor.matmul(
                        out=oacc[:, o0:512],
                        lhsT=xn[:, HD * i : HD * i + HD],
                        rhs=att[:, 512 * s + o0 : 512 * s + 512],
                        start=(mmpos[0] == 0),
                        stop=(mmpos[0] == n_total - 1),
                    )
                    mmpos[0] += 1

            waves = [
                (p, j, t, 2 * j + 2)
                for p in range(PAIRS)
                for j in range(QB)
                for t in range(2 * j + 2)
            ]

            load_pair(0)
            pend = []      # waves awaiting mm2 (pipeline depth 1)
            defr = []      # PWL waves' mm2, deferred to end of j
            flush = None   # end-of-j work emitted after next wave's scores
            out_sb = None
            oacc = None
            mmpos = None

            for (p, j, t, nwave) in waves:
                if t == 0:
                    if p + 1 < PAIRS and j == 2 and p + 1 not in state:
                        load_pair(p + 1)
                    if j == 0:
                        out_sb_next = outp.tile([HD, T], F32, name="osb", tag="osb")
                    else:
                        out_sb_next = out_sb
                    oacc_next = psum_o.tile([HD, 512], F32, name="oacc", tag="oacc")
                    mmpos_next = [0]

                xw, xn = state[p]
                i0 = 2 * t
                r0 = i0 - 4 * j
                tr = 256 if r0 == 2 else 0

                # --- score matmuls (2 k-tiles, co-executing PE row groups)
                S = psum_s.tile([128, 1024], F32, name="S", tag="S")
                for sl, (i, s, trs) in enumerate(((i0, 0, tr), (i0 + 1, 1, 0))):
                    bp = 64 * sl
                    nc.tensor.matmul(
                        out=S[:, 512 * s + trs : 512 * s + 512],
                        lhsT=xw[bp : bp + 64, _wxcol(i) : _wxcol(i) + 128],
                        rhs=xw[bp : bp + 64, 1024 * j + trs : 1024 * j + 512],
                        start=True,
                        stop=True,
                    )

                # --- end-of-previous-j flush sits here, AFTER this wave's
                # score matmuls, so the PE stays fed while the old j drains.
                if flush is not None:
                    fl_groups, fl_oacc, fl_osb, fl_j, fl_p, pair_end = flush
                    for g in fl_groups:
                        emit_mm2(g)
                    nc.vector.tensor_scalar_mul(
                        fl_osb[:, 512 * fl_j : 512 * fl_j + 512], fl_oacc, 1.0
                    )
                    if pair_end:
                        nc.sync.dma_start(out=out_d[fl_p], in_=fl_osb)
                    flush = None
                if t == 0:
                    out_sb = out_sb_next
                    oacc = oacc_next
                    mmpos = mmpos_next

                # --- sigmoid: scalar engine act or DVE PWL chain
                att = attnp.tile([128, 1024], BF16, name="att", tag="att")
                if (j, t) in OFFLOAD:
                    zc = pwlp.tile([128, 1024], BF16, name="zc", tag="zc")
                    t1 = pwlp.tile([128, 1024], BF16, name="t1", tag="t1")
                    t2 = pwlp.tile([128, 1024], BF16, name="t2", tag="t2")
                    nc.vector.tensor_scalar(out=zc, in0=S[:], scalar1=1.0,
                                            scalar2=None, op0=MULT)
                    nc.vector.tensor_scalar(out=t1, in0=zc, scalar1=PA1,
                                            scalar2=PB1, op0=MULT, op1=MIN)
                    nc.vector.tensor_scalar(out=t2, in0=zc, scalar1=PA2,
                                            scalar2=PB2, op0=MULT, op1=MIN)
                    nc.vector.tensor_scalar(out=t2, in0=t2, scalar1=-PB2,
                                            scalar2=0.5, op0=MAX, op1=ADD)
                    nc.vector.scalar_tensor_tensor(
                        out=att, in0=t1, scalar=-PB1, in1=t2, op0=MAX, op1=ADD)
                else:
                    nc.scalar.activation(out=att[:, tr:1024], in_=S[:, tr:1024],
                                         func=SIG)

                # --- causal staircase on the wave's diagonal blocks (gpsimd)
                if r0 >= 0:
                    bi0 = r0  # blocks at (slot 0, r0) and (slot 1, r0+1)
                    v = att[:].rearrange("p (n c) -> p n c", c=128)
                    sel = v[:, bi0 : bi0 + 6 : 5, :]
                    nc.gpsimd.affine_select(
                        out=sel, in_=sel, pattern=[[0, 2], [1, 128]],
                        compare_op=GE, fill=zero_reg, base=0,
                        channel_multiplier=-1)

                # --- pipeline: emit mm2 of the previous wave
                if pend:
                    emit_mm2(pend.pop(0))
                g = (p, j, att, i0, oacc, mmpos)
                if (j, t) in OFFLOAD:
                    defr.append(g)
                else:
                    pend.append(g)

                if t == nwave - 1:
                    flush = (pend + defr, oacc, out_sb, j, p, j == QB - 1)
                    pend = []
                    defr = []

            # final flush
            fl_groups, fl_oacc, fl_osb, fl_j, fl_p, pair_end = flush
            for g in fl_groups:
                emit_mm2(g)
            nc.vector.tensor_scalar_mul(
                fl_osb[:, 512 * fl_j : 512 * fl_j + 512], fl_oacc, 1.0)
            nc.sync.dma_start(out=out_d[fl_p], in_=fl_osb)
    nc.compile()
    return nc


_CACHE: dict = {}


def _get_nc():
    if "nc" not in _CACHE:
        _CACHE["nc"] = build()
    return _CACHE["nc"]


def _make_in_maps(x, bv_q, bv_k, bv_v):
    x = np.asarray(x, dtype=np.float32)
    bv_q = np.asarray(bv_q, dtype=np.float32)
    bv_k = np.asarray(bv_k, dtype=np.float32)
    bv_v = np.asarray(bv_v, dtype=np.float32)
    w = 0.5 * np.sign(bv_q) * np.sign(bv_k)
    sv = np.sign(bv_v)

    in_maps = []
    for c in range(N_CORES):
        xw = np.empty((PAIRS, 128, 2 * T), np.float32)
        xn = np.empty((PAIRS, 128, KT * HD), ml_dtypes.bfloat16)
        for p in range(PAIRS):
            g = PAIRS * c + p
            b, h = divmod(g, H)
            xs = x[b, :, HD * h : HD * h + HD]  # [T, HD]
            xsT_r = _round_fp32r(xs.T)          # [HD, T]
            wxT = xsT_r * w[h][:, None]         # exact (+-0.5)
            for ch in range(4):
                cs = slice(512 * ch, 512 * ch + 512)
                xw[p, 0:HD, 1024 * ch : 1024 * ch + 512] = xsT_r[:, cs]
                xw[p, HD:128, 1024 * ch : 1024 * ch + 512] = xsT_r[:, cs]
                xw[p, 0:HD, 1024 * ch + 512 : 1024 * ch + 1024] = wxT[:, cs]
                xw[p, HD:128, 1024 * ch + 512 : 1024 * ch + 1024] = wxT[:, cs]
            xsv = xs * sv[h][None, :]
            xn[p] = (
                xsv.reshape(KT, 128, HD).transpose(1, 0, 2).reshape(128, KT * HD)
            )
        in_maps.append({"xw": xw, "xn": xn})
    return in_maps


def _assemble(results):
    out = np.empty((B, T, D), np.float32)
    for c in range(N_CORES):
        oT = results[c]["outT"]  # [PAIRS, HD, T]
        for p in range(PAIRS):
            g = PAIRS * c + p
            b, h = divmod(g, H)
            out[b, :, HD * h : HD * h + HD] = oT[p].T
    return out


def _run(x, bv_q, bv_k, bv_v, **spmd_kwargs):
    in_maps = _make_in_maps(x, bv_q, bv_k, bv_v)
    res = run_bass_kernel_spmd(
        _get_nc(), in_maps, core_ids=list(range(N_CORES)), **spmd_kwargs
    )
    return _assemble(res.results), res


def kernel(x, bv_q, bv_k, bv_v):
    out, _ = _run(x, bv_q, bv_k, bv_v)
    return out
